# revision 3
# baseline (speedup 1.0000x reference)
"""Trainium2 Bass kernel for nn_Encoder_Cross — feature-major pipeline.

Data-parallel over batch: 8 batches -> 8 NeuronCores, weights replicated.

All activations live feature-major (FM): [128(d%128), 4(d//128), T] bf16,
token t = v*256 + p on the free dim.  Every linear layer uses weights as the
stationary operand and FM activations as the moving operand, producing FM
output directly — no inter-layer transposes, no DRAM round trip.  LayerNorm
stats are N=1 matmuls on PE (feature-dim column sums); the per-token
scale/offset rows are broadcast across partitions on GPSIMD.  The var_ccc
gather attention is dense masked [128,128] attention per 4-p-slot group with
a host-built log-count mask.

Algebraic rewrites (biases are zero, LN affine params are identity):
  patch scores  s  = x^T k'   with k'  = Wq1^T k1
  patch output  a1 = kW-weighted sums  with kW = Wo1^T k1
  cross scores  s2 = xc^T u   with u = (Wk2^T Wq2)^T-ish per group
  cross output  a2o: k2W = Wo2 k2; lo half passes through directly
"""
import sys

sys.path.insert(0, "/opt/trn_rl_repo")

import os

import numpy as np
import ml_dtypes

import concourse.bass as bass
import concourse.mybir as mybir
import concourse.tile as tile
from concourse import bacc
from concourse.bass_utils import run_bass_kernel_spmd
from concourse.masks import make_identity

F32 = mybir.dt.float32
BF16 = mybir.dt.bfloat16
FP8 = mybir.dt.float8e4
I32 = mybir.dt.int32
FP8_FFN = True
FP8_SCALE = 16.0

B, V, P, D = 8, 32, 256, 512
PERIOD = 16
S = P // PERIOD          # 16 pooled slots per variable
NS = V * S               # 512 pooled slots total
PARTIAL = 2
N_REL = 8
LN_EPS = 1e-5
SCALE = 1.0 / float(np.sqrt(D))
H2 = 2 * D
N_CORES = 8
MAGIC = 0x5F3759DF
T = V * P                # 8192 tokens
NG = 16                  # 512-token groups
NA = 32                  # cross-attn groups (4 hi-p x 32 v)


def _newton_rsqrt(nc, pool, var_ap, eps, width):
    """rstd = 1/sqrt(var + eps) on DVE only.  var_ap: [128, width] f32."""
    Alu = mybir.AluOpType
    ve = pool.tile([128, width], F32, tag="nw_ve", name="nw_ve")
    nc.vector.tensor_scalar(out=ve, in0=var_ap, scalar1=float(eps), scalar2=None,
                            op0=Alu.add)
    magic = pool.tile([128, width], I32, tag="nw_mg", name="nw_mg")
    nc.vector.memset(magic, MAGIC)
    half_i = pool.tile([128, width], I32, tag="nw_hi", name="nw_hi")
    nc.vector.tensor_scalar(out=half_i, in0=ve.bitcast(I32), scalar1=1, scalar2=None,
                            op0=Alu.logical_shift_right)
    y = pool.tile([128, width], F32, tag="nw_y", name="nw_y")
    nc.vector.tensor_tensor(out=y.bitcast(I32), in0=magic, in1=half_i,
                            op=Alu.subtract)
    u = pool.tile([128, width], F32, tag="nw_u", name="nw_u")
    w = pool.tile([128, width], F32, tag="nw_w", name="nw_w")
    for _ in range(2):
        nc.vector.tensor_tensor(out=u, in0=y, in1=y, op=Alu.mult)
        nc.vector.scalar_tensor_tensor(out=w, in0=u, scalar=-0.5, in1=ve,
                                       op0=Alu.mult, op1=Alu.mult)
        nc.vector.scalar_tensor_tensor(out=y, in0=w, scalar=1.5, in1=y,
                                       op0=Alu.add, op1=Alu.mult)
    return y


def build_nc():
    nc = bacc.Bacc("TRN2", target_bir_lowering=False, debug=False,
                   num_devices=N_CORES)

    def din(name, shape, dt=BF16):
        return nc.dram_tensor(name, shape, dt, kind="ExternalInput").ap()

    x_d = din("x", [V, P, D], F32)
    c4_d = din("c4", [128, 128], F32)
    wpool_d = din("wpool", [P, S])
    wdt = FP8 if FP8_FFN else BF16
    wk1t_d = din("wk1t", [D, D])
    wq1r_d = din("wq1r", [D, D])
    wo1t_d = din("wo1t", [D, D])
    w1ft_d = din("w1ft", [D, H2], wdt)
    w2ft_d = din("w2ft", [H2, D], wdt)
    wk2t_d = din("wk2t", [D, D])
    wk2r_d = din("wk2r", [D, D])
    wq2r_d = din("wq2r", [D, D])
    wo2t_d = din("wo2t", [D, D])
    w3ft_d = din("w3ft", [D, H2], wdt)
    w4ft_d = din("w4ft", [H2, D], wdt)
    out_d = nc.dram_tensor("out", [V, P, D], F32, kind="ExternalOutput").ap()
    dbg = {}
    if os.environ.get("FMDBG"):
        for nm, shape, dt_ in [
                ("dbg_xsA", [128, 4, T], BF16),
                ("dbg_kfam", [128, 4, 5 * NS], BF16),
                ("dbg_x1", [128, 4, T], BF16),
                ("dbg_xc", [128, 4, T], BF16),
                ("dbg_e", [128, 4, 2 * 4096 + D], BF16),
                ("dbg_f", [128, 4, 2 * 4096], BF16),
                ("dbg_g", [128, 4, 2 * 4096], BF16),
                ("dbg_ab", [128, 128], F32),
                ("dbg_ln0s", [128, 4, 2], F32),
                ("dbg_ln0ab", [128, 4, 128], BF16),
                ("dbg_ln0bc", [128, 2, 512], BF16)]:
            dbg[nm] = nc.dram_tensor(nm, shape, dt_,
                                     kind="ExternalOutput").ap()

    with tile.TileContext(nc) as tc:
        _build_body(nc, tc, x_d, c4_d, wpool_d, wk1t_d, wq1r_d, wo1t_d,
                    w1ft_d, w2ft_d, wk2t_d, wk2r_d, wq2r_d, wo2t_d,
                    w3ft_d, w4ft_d, out_d, dbg)
    nc.compile()
    return nc


def _build_body(nc, tc, x_d, c4_d, wpool_d, wk1t_d, wq1r_d, wo1t_d,
                w1ft_d, w2ft_d, wk2t_d, wk2r_d, wq2r_d, wo2t_d,
                w3ft_d, w4ft_d, out_d, dbg=None):
    from contextlib import ExitStack

    def tap(name, ap):
        if dbg and name in dbg:
            nc.sync.dma_start(out=dbg[name], in_=ap)
    Alu = mybir.AluOpType
    Act = mybir.ActivationFunctionType
    Ax = mybir.AxisListType

    def load_wT(pool, dram_ap, dk, dout, name, dt_=BF16):
        t = pool.tile([128, dk // 128, dout], dt_, name=name)
        nc.sync.dma_start(out=t, in_=dram_ap.rearrange("(k p) d -> p k d", p=128))
        return t

    def cols(tile_ap, kc, pattern, base):
        """Strided column slice of a [128, 4, W] FM tile: free dims `pattern`
        (list of [step, num] in elements) at column offset `base` in chunk kc.
        """
        return bass.AP(tensor=tile_ap.tensor,
                       offset=tile_ap.offset + kc * tile_ap.ap[1][0] + base,
                       ap=[list(tile_ap.ap[0]), *[list(p) for p in pattern]])

    stack = ExitStack()
    with stack:
        persist = stack.enter_context(tc.tile_pool(name="persist", bufs=1))
        nwp = stack.enter_context(tc.tile_pool(name="newton", bufs=2))
        lnp = stack.enter_context(tc.tile_pool(name="lnp", bufs=2))
        bcp = stack.enter_context(tc.tile_pool(name="bcast", bufs=3))
        ps_ln = stack.enter_context(
            tc.tile_pool(name="psLN", bufs=1, space="PSUM"))
        ps_row = stack.enter_context(
            tc.tile_pool(name="psRow", bufs=1, space="PSUM"))

        ident = persist.tile([128, 128], BF16, name="ident")
        make_identity(nc, ident)
        identf = persist.tile([128, 128], F32, name="identf")
        make_identity(nc, identf)
        c4_sb = persist.tile([128, 128], F32, name="c4_sb")
        nc.sync.dma_start(out=c4_sb, in_=c4_d)
        ones_col = persist.tile([128, 1], BF16, name="ones_col")
        nc.vector.memset(ones_col, 1.0)

        # streaming FM activations: x -> x1_ln -> xc  (dies after phase F)
        xsp_stack = stack.enter_context(ExitStack())
        xsp = xsp_stack.enter_context(tc.tile_pool(name="xsp", bufs=1))
        xs = xsp.tile([128, 4, T], BF16, name="xs")
        # LN4 per-token scale/offset, token-tile-indexed (lo: j=v; hi: 32+a)
        a4_sb = persist.tile([128, 64], F32, name="a4_sb")
        b4_sb = persist.tile([128, 64], F32, name="b4_sb")

        def ln_core(tag, jx, jsq, apply_regions, a4cols=None,
                    op1_engine=None):
            """FM layernorm from token tiles.

            jx/jsq: list (len nj) of 4-elem lists of [128,128] bf16 APs
              (x and x^2 token tiles, one per feature chunk kc).
            apply_regions: list of (src_ap, out_ap, lo, w): computes
              out = src * a_bc[:, lo:lo+w] + b_bc[:, lo:lo+w].
              If None: store per-token a/b into a4/b4[:, c0:c1] (a4cols).
            """
            nj = len(jx)
            sxy_full = ps_ln.tile([128, 264], F32, tag="sxy",
                                  name=f"{tag}_sxy")
            sxy = sxy_full[:, 0:8].rearrange("p (a b) -> p a b",
                                             b=2)[:, 0:nj, :]
            for j in range(nj):
                for si, jt in ((0, jx[j]), (1, jsq[j])):
                    for kc in range(4):
                        nc.tensor.matmul(sxy[:, j, si:si + 1], jt[kc],
                                         ones_col,
                                         start=(kc == 0), stop=(kc == 3))
            sums = lnp.tile([128, nj, 2], F32, tag=f"sums{nj}",
                            name=f"{tag}_s")
            nc.vector.tensor_copy(out=sums, in_=sxy[:, 0:nj, :])
            if dbg and tag == "ln0":
                nc.sync.dma_start(out=dbg["dbg_ln0s"], in_=sums)
            sx = sums[:, :, 0]
            sx2 = sums[:, :, 1]
            t2 = lnp.tile([128, nj], F32, tag=f"t2{nj}", name=f"{tag}_t2")
            nc.vector.tensor_tensor(out=t2, in0=sx, in1=sx, op=Alu.mult)
            ve = lnp.tile([128, nj], F32, tag=f"ve{nj}", name=f"{tag}_ve")
            nc.vector.scalar_tensor_tensor(out=ve, in0=sx2, scalar=float(D),
                                           in1=t2, op0=Alu.mult,
                                           op1=Alu.subtract)
            y = _newton_rsqrt(nc, nwp, ve, LN_EPS * D * D, nj)
            if apply_regions is None:
                c0, c1 = a4cols
                nc.vector.tensor_scalar(out=a4_sb[:, c0:c1], in0=y,
                                        scalar1=float(D), scalar2=None,
                                        op0=Alu.mult)
                nc.vector.scalar_tensor_tensor(out=b4_sb[:, c0:c1], in0=sx,
                                               scalar=-1.0, in1=y,
                                               op0=Alu.mult, op1=Alu.mult)
                return
            mu_sb = lnp.tile([128, nj], BF16, tag=f"mu{nj}",
                             name=f"{tag}_mu")
            nc.vector.tensor_scalar(out=mu_sb, in0=sx, scalar1=1.0 / D,
                                    scalar2=None, op0=Alu.mult)
            a_sb = lnp.tile([128, nj], F32, tag=f"a_sb{nj}", name=f"{tag}_a")
            nc.vector.tensor_scalar(out=a_sb, in0=y, scalar1=float(D),
                                    scalar2=None, op0=Alu.mult)
            murow = sxy_full.bitcast(BF16)[0:1, 16:16 + 512].rearrange(
                "p (a b) -> p a b", b=128)
            arow = ps_row.tile([1, 4, 128], F32, tag="arow",
                               name=f"{tag}_ar")
            for j in range(nj):
                nc.tensor.transpose(murow[0:1, j, :], mu_sb[:, j:j + 1],
                                    ident)
                nc.tensor.transpose(arow[0:1, j, :], a_sb[:, j:j + 1],
                                    identf)
            W = nj * 128
            mu_row_sb = lnp.tile([1, 4, 128], BF16, tag="mursb",
                                 name=f"{tag}_mursb")
            a_row_sb = lnp.tile([1, 4, 128], F32, tag="arsb",
                                name=f"{tag}_arsb")
            nc.scalar.copy(out=mu_row_sb[0:1, 0:nj, :],
                           in_=murow[0:1, 0:nj, :])
            nc.scalar.copy(out=a_row_sb[0:1, 0:nj, :], in_=arow[0:1, 0:nj, :])
            mu_bc = bcp.tile([128, W], BF16, tag=f"mu_bc{W}",
                             name=f"{tag}_mubc")
            a_bc = bcp.tile([128, W], F32, tag=f"a_bc{W}", name=f"{tag}_abc")
            nc.gpsimd.partition_broadcast(mu_bc, mu_row_sb[0:1, 0:nj, :])
            nc.gpsimd.partition_broadcast(a_bc, a_row_sb[0:1, 0:nj, :])
            for (src_ap, dst, lo, w) in apply_regions:
                scr = bcp.tile([128, w], BF16, tag=f"scr{w}",
                               name=f"{tag}_scr")
                mu_sl = mu_bc[:, lo:lo + w]
                a_sl = a_bc[:, lo:lo + w]
                scr_v = scr
                if len(src_ap.shape) == 3:
                    b2 = src_ap.shape[2]
                    mu_sl = mu_sl.rearrange("p (a b) -> p a b", b=b2)
                    a_sl = a_sl.rearrange("p (a b) -> p a b", b=b2)
                    scr_v = scr.rearrange("p (a b) -> p a b", b=b2)
                nc.vector.tensor_tensor(out=scr_v, in0=src_ap, in1=mu_sl,
                                        op=Alu.subtract)
                eng1 = op1_engine or nc.gpsimd
                eng1.tensor_tensor(out=dst, in0=scr_v, in1=a_sl, op=Alu.mult)

        # ================= PHASE 1 =================
        with ExitStack() as ph1:
            w1p = ph1.enter_context(tc.tile_pool(name="w1p", bufs=1))
            wpool_sb = w1p.tile([128, 2, S], BF16, name="wpool_sb")
            nc.sync.dma_start(out=wpool_sb,
                              in_=wpool_d.rearrange("(k p) s -> p k s", p=128))
            wk1t = load_wT(w1p, wk1t_d, D, D, "wk1t")
            wq1r = load_wT(w1p, wq1r_d, D, D, "wq1r")
            wo1t = load_wT(w1p, wo1t_d, D, D, "wo1t")
            wdt = FP8 if FP8_FFN else BF16
            w1ft = load_wT(w1p, w1ft_d, D, H2, "w1ft", wdt)
            w2ft = load_wT(w1p, w2ft_d, H2, D, "w2ft", wdt)

            kfam = ph1.enter_context(tc.tile_pool(name="kfam", bufs=1))
            xp_ln = kfam.tile([128, 4, NS], BF16, name="xp_ln")
            k1_sb = kfam.tile([128, 4, NS], BF16, name="k1_sb")
            kq_sb = kfam.tile([128, 4, NS], BF16, name="kq_sb")
            kW_sb = kfam.tile([128, 4, NS], BF16, name="kW_sb")



            # --- A: load x, cast bf16, pool, transpose to FM ---
            with ExitStack() as sAB:
                work = sAB.enter_context(tc.tile_pool(name="work1", bufs=8))
                workb = sAB.enter_context(tc.tile_pool(name="workb", bufs=10))
                xpp = sAB.enter_context(tc.tile_pool(name="xpp", bufs=1))
                xp_all = xpp.tile([128, 4, NS], F32, name="xp_all")
                xp_bf = xpp.tile([128, 4, NS], BF16, name="xp_bf")
                xp_sq = xpp.tile([128, 4, NS], BF16, name="xp_sq")
                ps_a = sAB.enter_context(tc.tile_pool(name="psA", bufs=2,
                                                      space="PSUM"))
                ps_tp = sAB.enter_context(tc.tile_pool(name="psTp", bufs=2,
                                                       space="PSUM"))
                for v in range(V):
                    xv = work.tile([128, 2, D], F32, tag="xv", name="xv")
                    nc.sync.dma_start(
                        out=xv,
                        in_=x_d[v].rearrange("(c p) d -> p c d", p=128))
                    xb = workb.tile([128, 2, D], BF16, tag="xb", name="xb")
                    nc.scalar.copy(out=xb, in_=xv)
                    xp_ps = ps_a.tile([128, 4, S], F32, tag="xp_ps",
                                      name="xp_ps")
                    for dc in range(4):
                        for c in range(2):
                            nc.tensor.matmul(
                                xp_ps[:, dc, :],
                                xb[:, c, dc * 128:(dc + 1) * 128],
                                wpool_sb[:, c, :],
                                start=(c == 0), stop=(c == 1))
                    nc.vector.tensor_copy(
                        out=xp_all[:, :, v * S:(v + 1) * S], in_=xp_ps)
                    # FM conversion via PE transposes (tp[dc, c] = xb chunk^T)
                    tp_ps = ps_tp.tile([128, 8, 128], BF16, tag="tp",
                                       name="tp_ps")
                    for c in range(2):
                        for dc in range(4):
                            nc.tensor.transpose(
                                tp_ps[:, c * 4 + dc, :],
                                xb[:, c, dc * 128:(dc + 1) * 128], ident)
                    for c in range(2):
                        b0 = v * 256 + c * 128
                        nc.vector.tensor_copy(
                            out=xs[:, :, b0:b0 + 128],
                            in_=tp_ps[:, c * 4:(c + 1) * 4, :])

                # --- B: LN0 + k-family ---
                nc.scalar.copy(out=xp_bf, in_=xp_all)
                nc.scalar.activation(out=xp_sq, in_=xp_bf, func=Act.Square)
                jx0 = [[xp_bf[:, kc, j * 128:(j + 1) * 128] for kc in range(4)]
                       for j in range(4)]
                jq0 = [[xp_sq[:, kc, j * 128:(j + 1) * 128] for kc in range(4)]
                       for j in range(4)]
                ln_core("ln0", jx0, jq0,
                        [(xp_bf[:, kc, :], xp_ln[:, kc, :], 0, NS)
                         for kc in range(4)])

                ps_k = sAB.enter_context(tc.tile_pool(name="psK", bufs=1,
                                                      space="PSUM"))
                for (wt, src, dst, nm) in ((wk1t, xp_ln, k1_sb, "k1"),
                                           (wq1r, k1_sb, kq_sb, "kq"),
                                           (wo1t, k1_sb, kW_sb, "kW")):
                    for half in range(2):
                        kps = ps_k.tile([128, 2, NS], F32, tag="kps",
                                        name=f"kps_{nm}")
                        for mi in range(2):
                            m = half * 2 + mi
                            for kc in range(4):
                                nc.tensor.matmul(
                                    kps[:, mi, :],
                                    wt[:, kc, m * 128:(m + 1) * 128],
                                    src[:, kc, :],
                                    start=(kc == 0), stop=(kc == 3))
                        nc.scalar.copy(out=dst[:, half * 2:half * 2 + 2, :],
                                       in_=kps)
                if dbg:
                    for i, tl in enumerate((xp_bf, xp_ln, k1_sb, kq_sb,
                                            kW_sb)):
                        nc.sync.dma_start(
                            out=dbg["dbg_kfam"][:, :, i * NS:(i + 1) * NS],
                            in_=tl)

            tap("dbg_xsA", xs)

            # --- C+D: patch attention, LN1, FFN1, LN2 per group ---
            with ExitStack() as sC:
                ps_s = sC.enter_context(tc.tile_pool(name="psS", bufs=1,
                                                     space="PSUM"))
                ps_at = sC.enter_context(tc.tile_pool(name="psAT", bufs=1,
                                                      space="PSUM"))
                ps_o1 = sC.enter_context(tc.tile_pool(name="psO1", bufs=1,
                                                      space="PSUM"))
                ps_h = sC.enter_context(tc.tile_pool(name="psH", bufs=2,
                                                     space="PSUM"))
                ps_y = sC.enter_context(tc.tile_pool(name="psY", bufs=1,
                                                     space="PSUM"))
                sm_p = sC.enter_context(tc.tile_pool(name="sm1", bufs=3))
                x1p = sC.enter_context(tc.tile_pool(name="x1p", bufs=2))
                hp = sC.enter_context(tc.tile_pool(name="hp", bufs=2))
                x2p = sC.enter_context(tc.tile_pool(name="x2p", bufs=2))
                x1t = {}
                x2t = {}

                def attn_stage(g):
                    gb = g * 512
                    x1pre = x1p.tile([128, 4, 512], BF16, tag="x1pre",
                                     name="x1pre")
                    x1sq = x1p.tile([128, 4, 512], BF16, tag="x1sq",
                                    name="x1sq")
                    x1t[g] = (x1pre, x1sq)
                    for vi in range(2):
                        v = 2 * g + vi
                        base = v * 256
                        s_ps = ps_s.tile([128, 2, S], F32, tag="s_ps",
                                         name="s_ps")
                        for c in range(2):
                            for kc in range(4):
                                nc.tensor.matmul(
                                    s_ps[:, c, :],
                                    xs[:, kc, base + c * 128:base + c * 128 + 128],
                                    kq_sb[:, kc, v * S:(v + 1) * S],
                                    start=(kc == 0), stop=(kc == 3))
                        es = sm_p.tile([128, 2, S], BF16, tag="es", name="es")
                        nc.scalar.activation(out=es, in_=s_ps, func=Act.Exp,
                                             scale=SCALE)
                        ssum = sm_p.tile([128, 2, 1], F32, tag="ssum",
                                         name="ssum")
                        nc.vector.tensor_reduce(out=ssum, in_=es, axis=Ax.X,
                                                op=Alu.add)
                        rs = sm_p.tile([128, 2, 1], F32, tag="rs", name="rs")
                        nc.vector.reciprocal(out=rs, in_=ssum)
                        attn = sm_p.tile([128, 2, S], BF16, tag="attn",
                                         name="attn")
                        for c in range(2):
                            nc.vector.tensor_scalar(out=attn[:, c, :],
                                                    in0=es[:, c, :],
                                                    scalar1=rs[:, c, :],
                                                    scalar2=None, op0=Alu.mult)
                        atk = ps_at.tile([16, 6, 128], BF16, tag="atk",
                                         name="atk")
                        for c in range(2):
                            nc.tensor.transpose(atk[:, 4 + c, :],
                                                attn[:, c, :], ident)
                        for kc in range(4):
                            nc.tensor.transpose(atk[:, kc, :],
                                                kW_sb[:, kc, v * S:(v + 1) * S],
                                                ident)
                        atk_sb = sm_p.tile([16, 6, 128], BF16, tag="atk_sb",
                                           name="atk_sb")
                        nc.vector.tensor_copy(out=atk_sb, in_=atk)
                        rhsT = atk_sb[:, 4:6, :].rearrange("s c p -> s (c p)")
                        for half in range(2):
                            o1_ps = ps_o1.tile([128, 2, 256], F32, tag="o1",
                                               name="o1_ps")
                            for di in range(2):
                                dc = half * 2 + di
                                nc.tensor.matmul(o1_ps[:, di, :],
                                                 atk_sb[:, dc, :], rhsT,
                                                 start=True, stop=True)
                            h0 = half * 2
                            nc.vector.tensor_tensor(
                                out=x1pre[:, h0:h0 + 2,
                                          vi * 256:(vi + 1) * 256],
                                in0=xs[:, h0:h0 + 2, base:base + 256],
                                in1=o1_ps, op=Alu.add)
                    nc.vector.tensor_tensor(out=x1sq, in0=x1pre, in1=x1pre,
                                            op=Alu.mult)

                def ln1_stage(g):
                    gb = g * 512
                    x1pre, x1sq = x1t.pop(g)
                    jx = [[x1pre[:, kc, j * 128:(j + 1) * 128]
                           for kc in range(4)] for j in range(4)]
                    jq = [[x1sq[:, kc, j * 128:(j + 1) * 128]
                           for kc in range(4)] for j in range(4)]
                    ln_core(f"ln1_{g}", jx, jq,
                            [(x1pre[:, kc, :], xs[:, kc, gb:gb + 512], 0, 512)
                             for kc in range(4)])

                def ffn1_stage(g):
                    gb = g * 512
                    if FP8_FFN:
                        x1f8 = hp.tile([128, 4, 512], FP8, tag="x1f8",
                                       name="x1f8")
                        nc.scalar.copy(out=x1f8, in_=xs[:, :, gb:gb + 512])
                        h_sb = hp.tile([128, 8, 512], FP8, tag="h_sb",
                                       name="h_sb")
                        for m in range(8):
                            h_ps = ps_h.tile([128, 512], F32, tag="h_ps",
                                             name="h_ps")
                            for kd in range(2):
                                nc.tensor.matmul(
                                    h_ps,
                                    w1ft[:, 2 * kd:2 * kd + 2,
                                         m * 128:(m + 1) * 128],
                                    x1f8[:, 2 * kd:2 * kd + 2, :],
                                    start=(kd == 0), stop=(kd == 1),
                                    perf_mode=mybir.MatmulPerfMode.DoubleRow)
                            nc.scalar.activation(out=h_sb[:, m, :], in_=h_ps,
                                                 func=Act.Gelu,
                                                 scale=1.0 / FP8_SCALE)
                    else:
                        h_sb = hp.tile([128, 8, 512], BF16, tag="h_sb",
                                       name="h_sb")
                        for m in range(8):
                            h_ps = ps_h.tile([128, 512], F32, tag="h_ps",
                                             name="h_ps")
                            for kc in range(4):
                                nc.tensor.matmul(
                                    h_ps, w1ft[:, kc, m * 128:(m + 1) * 128],
                                    xs[:, kc, gb:gb + 512],
                                    start=(kc == 0), stop=(kc == 3))
                            nc.scalar.activation(out=h_sb[:, m, :], in_=h_ps,
                                                 func=Act.Gelu)
                    x2pre = x2p.tile([128, 4, 512], BF16, tag="x2pre",
                                     name="x2pre")
                    for dc in range(4):
                        y_ps = ps_y.tile([128, 512], F32, tag="y_ps",
                                         name="y_ps")
                        if FP8_FFN:
                            for kd in range(4):
                                nc.tensor.matmul(
                                    y_ps,
                                    w2ft[:, 2 * kd:2 * kd + 2,
                                         dc * 128:(dc + 1) * 128],
                                    h_sb[:, 2 * kd:2 * kd + 2, :],
                                    start=(kd == 0), stop=(kd == 3),
                                    perf_mode=mybir.MatmulPerfMode.DoubleRow)
                            nc.vector.scalar_tensor_tensor(
                                out=x2pre[:, dc, :], in0=y_ps,
                                scalar=1.0 / FP8_SCALE,
                                in1=xs[:, dc, gb:gb + 512],
                                op0=Alu.mult, op1=Alu.add)
                        else:
                            for hk in range(8):
                                nc.tensor.matmul(
                                    y_ps,
                                    w2ft[:, hk, dc * 128:(dc + 1) * 128],
                                    h_sb[:, hk, :],
                                    start=(hk == 0), stop=(hk == 7))
                            nc.vector.tensor_tensor(
                                out=x2pre[:, dc, :],
                                in0=xs[:, dc, gb:gb + 512],
                                in1=y_ps, op=Alu.add)
                    x2sq = x2p.tile([128, 4, 512], BF16, tag="x2sq",
                                    name="x2sq")
                    nc.vector.tensor_tensor(out=x2sq, in0=x2pre, in1=x2pre,
                                            op=Alu.mult)
                    x2t[g] = (x2pre, x2sq)

                def ln2_stage(g):
                    gb = g * 512
                    x2pre, x2sq = x2t.pop(g)
                    jx = [[x2pre[:, kc, j * 128:(j + 1) * 128]
                           for kc in range(4)] for j in range(4)]
                    jq = [[x2sq[:, kc, j * 128:(j + 1) * 128]
                           for kc in range(4)] for j in range(4)]
                    ln_core(f"ln2_{g}", jx, jq,
                            [(x2pre[:, kc, :], xs[:, kc, gb:gb + 512], 0, 512)
                             for kc in range(4)])

                for t in range(NG + 3):
                    if t < NG:
                        attn_stage(t)
                    if 0 <= t - 1 < NG:
                        ln1_stage(t - 1)
                    if 0 <= t - 2 < NG:
                        ffn1_stage(t - 2)
                    if 0 <= t - 3 < NG:
                        ln2_stage(t - 3)

        tap("dbg_xc", xs)

        # ================= PHASE 2 =================
        with ExitStack() as ph2:
            # lo tokens move to x3lo (col l = v*128 + p, p<128) at phase E;
            # hi tokens to x3hi (col = a*128 + 4v + pi) at phase F.
            strm = ph2.enter_context(tc.tile_pool(name="strm", bufs=1))
            x3lo = strm.tile([128, 4, 4096], BF16, name="x3lo")
            strm_hi = ph2.enter_context(tc.tile_pool(name="strmh", bufs=1))
            x3hi = strm_hi.tile([128, 4, 4096], BF16, name="x3hi")

            sEF = ph2.enter_context(ExitStack())
            big2 = sEF.enter_context(tc.tile_pool(name="big2", bufs=1))
            k2Whi = big2.tile([128, 4, 4096], BF16, name="k2Whi")
            Mt_sb = big2.tile([128, 4, D], BF16, name="Mt_sb")

            # --- E: Mt; per group: k2 -> k2W (split lo/hi); lo residual ---
            with ExitStack() as sE:
                w2p = sE.enter_context(tc.tile_pool(name="w2p", bufs=1))
                wk2t = load_wT(w2p, wk2t_d, D, D, "wk2t")
                wk2r = load_wT(w2p, wk2r_d, D, D, "wk2r")
                wq2r = load_wT(w2p, wq2r_d, D, D, "wq2r")
                wo2t = load_wT(w2p, wo2t_d, D, D, "wo2t")
                ps_e = sE.enter_context(tc.tile_pool(name="psE", bufs=3,
                                                     space="PSUM"))
                k2p = sE.enter_context(tc.tile_pool(name="k2p", bufs=1))
                for half in range(2):
                    mt_ps = ps_e.tile([128, 2, 512], F32, tag="eps",
                                      name="mt_ps")
                    for ji in range(2):
                        jm = half * 2 + ji
                        for ac in range(4):
                            nc.tensor.matmul(
                                mt_ps[:, ji, :],
                                wk2r[:, ac, jm * 128:(jm + 1) * 128],
                                wq2r[:, ac, :],
                                start=(ac == 0), stop=(ac == 3))
                    nc.scalar.copy(out=Mt_sb[:, half * 2:half * 2 + 2, :],
                                   in_=mt_ps)
                lop2 = [[256, 2], [1, 128]]
                for g in range(NG):
                    gb = g * 512
                    k2_sb = k2p.tile([128, 4, 512], BF16, tag="k2_sb",
                                     name="k2_sb")
                    for half in range(2):
                        kps = ps_e.tile([128, 2, 512], F32, tag="eps",
                                        name="k2_ps")
                        for mi in range(2):
                            m = half * 2 + mi
                            for kc in range(4):
                                nc.tensor.matmul(
                                    kps[:, mi, :],
                                    wk2t[:, kc, m * 128:(m + 1) * 128],
                                    xs[:, kc, gb:gb + 512],
                                    start=(kc == 0), stop=(kc == 3))
                        nc.scalar.copy(out=k2_sb[:, half * 2:half * 2 + 2, :],
                                       in_=kps)
                    for half in range(2):
                        kps = ps_e.tile([128, 2, 512], F32, tag="eps",
                                        name="k2W_ps")
                        for mi in range(2):
                            m = half * 2 + mi
                            for kc in range(4):
                                nc.tensor.matmul(
                                    kps[:, mi, :],
                                    wo2t[:, kc, m * 128:(m + 1) * 128],
                                    k2_sb[:, kc, :],
                                    start=(kc == 0), stop=(kc == 3))
                        h0 = half * 2
                        src_lo = bass.AP(
                            tensor=kps.tensor, offset=kps.offset,
                            ap=[list(kps.ap[0]), [512, 2], [256, 2], [1, 128]])

                        nc.scalar.copy(
                            out=x3lo[:, h0:h0 + 2, g * 256:(g + 1) * 256]
                            .rearrange("p m (a b) -> p m a b", b=128),
                            in_=src_lo)
                        for vp in range(2):
                            src_hi = bass.AP(
                                tensor=kps.tensor,
                                offset=kps.offset + vp * 256 + 128,
                                ap=[list(kps.ap[0]), [512, 2], [4, 32],
                                    [1, 4]])
                            dst_hi = bass.AP(
                                tensor=k2Whi.tensor,
                                offset=k2Whi.offset + h0 * 4096
                                + 8 * g + 4 * vp,
                                ap=[list(k2Whi.ap[0]), [4096, 2], [128, 32],
                                    [1, 4]])
                            nc.scalar.copy(out=dst_hi, in_=src_hi)
                    # lo passthrough residual: x3lo += xc_lo
                    for kc in range(4):
                        dst = x3lo[:, kc, g * 256:(g + 1) * 256].rearrange(
                            "p (a b) -> p a b", b=128)
                        nc.vector.tensor_tensor(
                            out=dst, in0=dst,
                            in1=cols(xs, kc, lop2, gb), op=Alu.add)

            if dbg:
                nc.sync.dma_start(out=dbg["dbg_e"][:, :, 0:4096], in_=x3lo)
                nc.sync.dma_start(out=dbg["dbg_e"][:, :, 4096:8192],
                                  in_=k2Whi)
                nc.sync.dma_start(out=dbg["dbg_e"][:, :, 8192:8704],
                                  in_=Mt_sb)

            # --- F: cross attention per attn-group + LN3 ---
            with ExitStack() as sF:
                ps_u = sF.enter_context(tc.tile_pool(name="psU", bufs=1,
                                                     space="PSUM"))
                ps_s2 = sF.enter_context(tc.tile_pool(name="psS2", bufs=1,
                                                      space="PSUM"))
                ps_km = sF.enter_context(tc.tile_pool(name="psKM", bufs=2,
                                                      space="PSUM"))
                ps_o2 = sF.enter_context(tc.tile_pool(name="psO2", bufs=2,
                                                      space="PSUM"))
                sm2 = sF.enter_context(tc.tile_pool(name="sm2", bufs=3))
                x3p = sF.enter_context(tc.tile_pool(name="x3p", bufs=1))
                ftile = {}

                def f_stageA(a):
                    hb = 128 + 4 * a
                    qpat = [[256, 32], [1, 4]]
                    xq_sb = sm2.tile([128, 4, 128], BF16, tag="xq_sb",
                                     name="xq_sb", bufs=4)
                    for kc in range(4):
                        nc.vector.tensor_copy(
                            out=xq_sb[:, kc, :].rearrange(
                                "p (a b) -> p a b", b=4),
                            in_=cols(xs, kc, qpat, hb))
                    u_ps = ps_u.tile([128, 4, 128], F32, tag="u_ps",
                                     name="u_ps")
                    for ic in range(4):
                        for jc in range(4):
                            nc.tensor.matmul(
                                u_ps[:, ic, :],
                                Mt_sb[:, jc, ic * 128:(ic + 1) * 128],
                                xq_sb[:, jc, :],
                                start=(jc == 0), stop=(jc == 3))
                    u_sb = sm2.tile([128, 4, 128], BF16, tag="u_sb",
                                    name="u_sb", bufs=3)
                    nc.scalar.copy(out=u_sb, in_=u_ps)
                    ftile[("xq", a)] = xq_sb
                    ftile[("u", a)] = u_sb

                def f_stageB(a):
                    xq_sb = ftile.pop(("xq", a))
                    u_sb = ftile.pop(("u", a))
                    s2_ps = ps_s2.tile([128, 128], F32, tag="s2", name="s2")
                    for ic in range(4):
                        nc.tensor.matmul(s2_ps, xq_sb[:, ic, :],
                                         u_sb[:, ic, :],
                                         start=(ic == 0), stop=(ic == 3))
                    w4log = sm2.tile([128, 128], F32, tag="w4log",
                                     name="w4log")
                    nc.vector.scalar_tensor_tensor(out=w4log, in0=s2_ps,
                                                   scalar=SCALE, in1=c4_sb,
                                                   op0=Alu.mult, op1=Alu.add)
                    es2 = sm2.tile([128, 128], BF16, tag="es2", name="es2")
                    nc.scalar.activation(out=es2, in_=w4log, func=Act.Exp)
                    sm = sm2.tile([128, 1], F32, tag="sm", name="sm")
                    nc.vector.tensor_reduce(out=sm, in_=es2, axis=Ax.X,
                                            op=Alu.add)
                    rs2 = sm2.tile([128, 1], F32, tag="rs2", name="rs2")
                    nc.vector.reciprocal(out=rs2, in_=sm)
                    w4 = sm2.tile([128, 128], BF16, tag="w4", name="w4",
                                  bufs=3)
                    nc.vector.tensor_scalar(out=w4, in0=es2, scalar1=rs2,
                                            scalar2=None, op0=Alu.mult)
                    ftile[("w4", a)] = w4

                def f_stageC(a):
                    w4 = ftile.pop(("w4", a))
                    kmp = ps_km.tile([128, 5, 128], BF16, tag="kmp",
                                     name="kmp")
                    nc.tensor.transpose(kmp[:, 4, :], w4, ident)
                    for kc in range(4):
                        nc.tensor.transpose(kmp[:, kc, :],
                                            k2Whi[:, kc, a * 128:(a + 1) * 128],
                                            ident)
                    km_sb = sm2.tile([128, 5, 128], BF16, tag="km_sb",
                                     name="km_sb", bufs=3)
                    nc.scalar.copy(out=km_sb, in_=kmp)
                    ftile[("km", a)] = km_sb

                def f_stageD(a):
                    hb = 128 + 4 * a
                    qpat = [[256, 32], [1, 4]]
                    km_sb = ftile.pop(("km", a))
                    o2_ps = ps_o2.tile([128, 4, 128], F32, tag="o2",
                                       name="o2_ps")
                    for dc in range(4):
                        nc.tensor.matmul(o2_ps[:, dc, :], km_sb[:, dc, :],
                                         km_sb[:, 4, :], start=True,
                                         stop=True)
                    for dc in range(4):
                        nc.vector.tensor_tensor(
                            out=x3hi[:, dc, a * 128:(a + 1) * 128].rearrange(
                                "p (a b) -> p a b", b=4),
                            in0=cols(xs, dc, qpat, hb),
                            in1=o2_ps[:, dc, :].rearrange(
                                "p (a b) -> p a b", b=4), op=Alu.add)
                    if a % 4 == 3:
                        a0 = a - 3
                        hb0 = a0 * 128
                        x3s = x3p.tile([128, 4, 512], BF16, tag="x3sq",
                                       name="x3sq")
                        nc.scalar.activation(
                            out=x3s, in_=x3hi[:, :, hb0:hb0 + 512],
                            func=Act.Square)
                        jx = [[x3hi[:, kc, hb0 + j * 128:hb0 + (j + 1) * 128]
                               for kc in range(4)] for j in range(4)]
                        jq = [[x3s[:, kc, j * 128:(j + 1) * 128]
                               for kc in range(4)] for j in range(4)]
                        ln_core(f"ln3h_{a0}", jx, jq,
                                [(x3hi[:, kc, hb0:hb0 + 512],
                                  x3hi[:, kc, hb0:hb0 + 512], 0, 512)
                                 for kc in range(4)])

                for t in range(NA + 3):
                    if t < NA:
                        f_stageA(t)
                    if 0 <= t - 1 < NA:
                        f_stageB(t - 1)
                    if 0 <= t - 2 < NA:
                        f_stageC(t - 2)
                    if 0 <= t - 3 < NA:
                        f_stageD(t - 3)

                # LN3 lo on x3lo (contiguous)
                for g2 in range(8):
                    b0 = g2 * 512
                    x3s = x3p.tile([128, 4, 512], BF16, tag="x3sq",
                                   name="x3sql")
                    nc.scalar.activation(out=x3s, in_=x3lo[:, :, b0:b0 + 512],
                                         func=Act.Square)
                    jx = [[x3lo[:, kc, b0 + j * 128:b0 + (j + 1) * 128]
                           for kc in range(4)] for j in range(4)]
                    jq = [[x3s[:, kc, j * 128:(j + 1) * 128]
                           for kc in range(4)] for j in range(4)]
                    ln_core(f"ln3l_{g2}", jx, jq,
                            [(x3lo[:, kc, b0:b0 + 512],
                              x3lo[:, kc, b0:b0 + 512], 0, 512)
                             for kc in range(4)])
            if dbg:
                nc.sync.dma_start(out=dbg["dbg_f"][:, :, 0:4096], in_=x3lo)
                nc.sync.dma_start(out=dbg["dbg_f"][:, :, 4096:8192],
                                  in_=x3hi)
            sEF.close()

            # --- G: FFN2 + LN4 stats + output ---
            with ExitStack() as sG:
                wgp = sG.enter_context(tc.tile_pool(name="wgp", bufs=1))
                w3ft = load_wT(wgp, w3ft_d, D, H2, "w3ft", wdt)
                w4ft = load_wT(wgp, w4ft_d, H2, D, "w4ft", wdt)
                ps_h2 = sG.enter_context(tc.tile_pool(name="psH2", bufs=2,
                                                      space="PSUM"))
                ps_z = sG.enter_context(tc.tile_pool(name="psZ", bufs=2,
                                                     space="PSUM"))
                hp2 = sG.enter_context(tc.tile_pool(name="hp2", bufs=3))
                x4p = sG.enter_context(tc.tile_pool(name="x4p", bufs=2))

                def ffn2_group(xtile, jb, a4c0, tag):
                    if FP8_FFN:
                        x3f8 = hp2.tile([128, 4, 512], FP8, tag="x3f8",
                                        name="x3f8")
                        nc.scalar.copy(out=x3f8, in_=xtile[:, :, jb:jb + 512])
                        h_sb = hp2.tile([128, 8, 512], FP8, tag="h2sb",
                                        name="h2sb")
                        for m in range(8):
                            h_ps = ps_h2.tile([128, 512], F32, tag="h2ps",
                                              name="h2ps")
                            for kd in range(2):
                                nc.tensor.matmul(
                                    h_ps,
                                    w3ft[:, 2 * kd:2 * kd + 2,
                                         m * 128:(m + 1) * 128],
                                    x3f8[:, 2 * kd:2 * kd + 2, :],
                                    start=(kd == 0), stop=(kd == 1),
                                    perf_mode=mybir.MatmulPerfMode.DoubleRow)
                            nc.scalar.activation(out=h_sb[:, m, :], in_=h_ps,
                                                 func=Act.Gelu,
                                                 scale=1.0 / FP8_SCALE)
                    else:
                        h_sb = hp2.tile([128, 8, 512], BF16, tag="h2sb",
                                        name="h2sb")
                        for m in range(8):
                            h_ps = ps_h2.tile([128, 512], F32, tag="h2ps",
                                              name="h2ps")
                            for kc in range(4):
                                nc.tensor.matmul(
                                    h_ps, w3ft[:, kc, m * 128:(m + 1) * 128],
                                    xtile[:, kc, jb:jb + 512],
                                    start=(kc == 0), stop=(kc == 3))
                            nc.scalar.activation(out=h_sb[:, m, :], in_=h_ps,
                                                 func=Act.Gelu)
                    for dc in range(4):
                        z_ps = ps_z.tile([128, 512], F32, tag="z_ps",
                                         name="z_ps")
                        if FP8_FFN:
                            for kd in range(4):
                                nc.tensor.matmul(
                                    z_ps,
                                    w4ft[:, 2 * kd:2 * kd + 2,
                                         dc * 128:(dc + 1) * 128],
                                    h_sb[:, 2 * kd:2 * kd + 2, :],
                                    start=(kd == 0), stop=(kd == 3),
                                    perf_mode=mybir.MatmulPerfMode.DoubleRow)
                            nc.vector.scalar_tensor_tensor(
                                out=xtile[:, dc, jb:jb + 512], in0=z_ps,
                                scalar=1.0 / FP8_SCALE,
                                in1=xtile[:, dc, jb:jb + 512],
                                op0=Alu.mult, op1=Alu.add)
                        else:
                            for hk in range(8):
                                nc.tensor.matmul(
                                    z_ps,
                                    w4ft[:, hk, dc * 128:(dc + 1) * 128],
                                    h_sb[:, hk, :],
                                    start=(hk == 0), stop=(hk == 7))
                            nc.vector.tensor_tensor(
                                out=xtile[:, dc, jb:jb + 512],
                                in0=xtile[:, dc, jb:jb + 512],
                                in1=z_ps, op=Alu.add)
                    x4s = x4p.tile([128, 4, 512], BF16, tag="x4sq",
                                   name="x4sq")
                    nc.vector.tensor_tensor(out=x4s,
                                            in0=xtile[:, :, jb:jb + 512],
                                            in1=xtile[:, :, jb:jb + 512],
                                            op=Alu.mult)
                    jx = [[xtile[:, kc, jb + jj * 128:jb + (jj + 1) * 128]
                           for kc in range(4)] for jj in range(4)]
                    jq = [[x4s[:, kc, jj * 128:(jj + 1) * 128]
                           for kc in range(4)] for jj in range(4)]
                    ln_core(tag, jx, jq, None, a4cols=(a4c0, a4c0 + 4))

                for j in range(8):
                    ffn2_group(x3lo, j * 512, 4 * j, f"ln4l_{j}")
                for j in range(8):
                    ffn2_group(x3hi, j * 512, 32 + 4 * j, f"ln4h_{j}")

                if dbg:
                    nc.sync.dma_start(out=dbg["dbg_g"][:, :, 0:4096],
                                      in_=x3lo)
                    nc.sync.dma_start(out=dbg["dbg_g"][:, :, 4096:8192],
                                      in_=x3hi)
                    nc.sync.dma_start(out=dbg["dbg_ab"][:, 0:64], in_=a4_sb)
                    nc.sync.dma_start(out=dbg["dbg_ab"][:, 64:128], in_=b4_sb)

                # output: transpose to token-major, apply LN4, store
                ps_ot = sG.enter_context(tc.tile_pool(name="psOT", bufs=2,
                                                      space="PSUM"))
                outp = sG.enter_context(tc.tile_pool(name="outp", bufs=3))
                for v in range(V):   # lo tokens of v
                    ott = ps_ot.tile([128, 4, 128], BF16, tag="ott",
                                     name="ott")
                    for kc in range(4):
                        nc.tensor.transpose(ott[:, kc, :],
                                            x3lo[:, kc, v * 128:(v + 1) * 128],
                                            ident)
                    ostg = outp.tile([128, 512], BF16, tag="ostg",
                                     name="ostg")
                    nc.vector.tensor_scalar(
                        out=ostg, in0=ott.rearrange("p a b -> p (a b)"),
                        scalar1=a4_sb[:, v:v + 1], scalar2=b4_sb[:, v:v + 1],
                        op0=Alu.mult, op1=Alu.add)
                    nc.gpsimd.dma_start(out=out_d[v, 0:128, :], in_=ostg)
                for a in range(NA):  # hi tokens, attn-layout
                    ott = ps_ot.tile([128, 4, 128], BF16, tag="ott",
                                     name="otth")
                    for kc in range(4):
                        nc.tensor.transpose(ott[:, kc, :],
                                            x3hi[:, kc, a * 128:(a + 1) * 128],
                                            ident)
                    ostg = outp.tile([128, 512], BF16, tag="ostg",
                                     name="ostgh")
                    nc.vector.tensor_scalar(
                        out=ostg, in0=ott.rearrange("p a b -> p (a b)"),
                        scalar1=a4_sb[:, 32 + a:33 + a],
                        scalar2=b4_sb[:, 32 + a:33 + a],
                        op0=Alu.mult, op1=Alu.add)
                    dst = bass.AP(tensor=out_d.tensor,
                                  offset=out_d.offset + (128 + 4 * a) * D,
                                  ap=[[P * D, V], [D, 4], [1, D]])
                    nc.gpsimd.dma_start(out=dst, in_=ostg)


_NC_CACHE = None


def _get_nc():
    global _NC_CACHE
    if _NC_CACHE is None:
        _NC_CACHE = build_nc()
    return _NC_CACHE


def _prep_weights(inputs):
    bf = ml_dtypes.bfloat16

    def t(a):
        return np.ascontiguousarray(np.asarray(a, np.float32).T).astype(bf)

    def r(a):
        return np.ascontiguousarray(np.asarray(a, np.float32)).astype(bf)

    Wp = np.asarray(inputs["Wp"], np.float32)     # [1, P//PERIOD]
    wpool = np.zeros((P, S), np.float32)
    for p in range(P):
        wpool[p, p % PERIOD] = Wp[0, p // PERIOD]
    if FP8_FFN:
        f8 = ml_dtypes.float8_e4m3fn

        def tf8(a):
            return np.ascontiguousarray(
                np.asarray(a, np.float32).T * FP8_SCALE).astype(f8)
        ffn = dict(w1ft=tf8(inputs["W1f"]), w2ft=tf8(inputs["W2f"]),
                   w3ft=tf8(inputs["W3f"]), w4ft=tf8(inputs["W4f"]))
    else:
        ffn = dict(w1ft=t(inputs["W1f"]), w2ft=t(inputs["W2f"]),
                   w3ft=t(inputs["W3f"]), w4ft=t(inputs["W4f"]))
    return dict(
        wpool=wpool.astype(bf),
        wk1t=t(inputs["Wk1"]), wq1r=r(inputs["Wq1"]), wo1t=t(inputs["Wo1"]),
        wk2t=t(inputs["Wk2"]), wk2r=r(inputs["Wk2"]), wq2r=r(inputs["Wq2"]),
        wo2t=t(inputs["Wo2"]), **ffn,
    )


def kernel(**inputs):
    nc = _get_nc()
    w = _prep_weights(inputs)
    x = np.asarray(inputs["x"], np.float32)
    ccc = np.asarray(inputs["var_ccc"])
    in_maps = []
    for b in range(N_CORES):
        cnt = np.zeros((V, V), np.float32)
        for v in range(V):
            for n in range(N_REL):
                cnt[v, int(ccc[b, v, n])] += 1.0
        c4 = np.kron(cnt, np.eye(4, dtype=np.float32))  # [128,128], m=4v+pi
        c4 = np.where(c4 > 0, np.log(np.maximum(c4, 1e-9)), -1e30).astype(np.float32)
        in_maps.append({"x": np.ascontiguousarray(x[b]), "c4": c4, **w})
    res = run_bass_kernel_spmd(nc, in_maps, core_ids=list(range(N_CORES)))
    global LAST_RESULTS
    LAST_RESULTS = res.results
    out = np.stack([res.results[b]["out"] for b in range(N_CORES)], axis=0)
    return out.astype(np.float32)


LAST_RESULTS = None


if __name__ == "__main__":
    rng = np.random.default_rng(0)
    fake = dict(
        x=rng.standard_normal((B, V, P, D), dtype=np.float32),
        var_ccc=rng.integers(0, V, (B, V, N_REL)),
        Wp=rng.standard_normal((1, P // PERIOD)).astype(np.float32) * 0.02,
    )
    for nm in ["Wq1", "Wk1", "Wo1", "Wq2", "Wk2", "Wo2"]:
        fake[nm] = rng.standard_normal((D, D)).astype(np.float32) * 0.02
    fake["W1f"] = rng.standard_normal((H2, D)).astype(np.float32) * 0.02
    fake["W2f"] = rng.standard_normal((D, H2)).astype(np.float32) * 0.02
    fake["W3f"] = rng.standard_normal((H2, D)).astype(np.float32) * 0.02
    fake["W4f"] = rng.standard_normal((D, H2)).astype(np.float32) * 0.02
    o = kernel(**fake)
    print("out", o.shape, o.dtype, float(np.abs(o).max()))
